# revision 11
# baseline (speedup 1.0000x reference)
"""Trainium2 Bass kernel for DSSnetwork GNN message passing (8 NeuronCores).

Sharding: graphs are distributed across cores (16 graphs/core). All edges are
intra-subgraph (64-node blocks), so message passing becomes dense 64x64
adjacency matmuls built on-device from the edge lists; the only cross-core
traffic is a tiny [128,4] AllReduce per layer for global BatchNorm statistics.

Layout: node features live feature-partitioned (h^T = [128 feat, nodes]) in
SBUF for the whole kernel. BatchNorm stats are free-axis reductions; the
h2[node_idx] gather and x_sum scatter-mean are regular strided access patterns.
"""

import numpy as np

# Problem constants (hardcoded per contract; inputs are validated at runtime).
B = 128      # graphs
S = 8        # subgraphs per graph
N0 = 64      # nodes per subgraph
DEG = 16     # edges per node
L = 3        # GNN layers
D = 128      # feature dim
T = 10       # tasks
EPS = 1e-5
N_CORES = 8

_CACHE = {}


def build_nc(G, n_cores):
    """Build the SPMD Bass/Tile program for one core holding G graphs."""
    from contextlib import ExitStack

    import concourse.bacc as bacc
    import concourse.bass as bass
    import concourse.mybir as mybir
    import concourse.tile as tile

    f32 = mybir.dt.float32
    bf16 = mybir.dt.bfloat16
    i32 = mybir.dt.int32
    Alu = mybir.AluOpType
    Act = mybir.ActivationFunctionType

    NN = G * S * N0            # nodes per core
    NOR = G * N0               # original nodes per core
    NPAIR = NN // 128          # 128-node (2-subgraph) pairs
    NPAIR_OR = NOR // 128
    assert NN % 512 == 0
    WINS = [(w * 512, 512) for w in range(NN // 512)]
    if NOR >= 512:
        assert NOR % 512 == 0
        WINS_OR = [(w * 512, 512) for w in range(NOR // 512)]
    else:
        WINS_OR = [(0, NOR)]
    NG1 = float(n_cores * NN)      # global node count (BN1)
    NG2 = float(n_cores * NOR)     # global orig-node count (BN2)

    nc = bacc.Bacc("TRN2", num_devices=n_cores)

    # ---- I/O ----
    xT = nc.dram_tensor("xT", [128, NN], f32, kind="ExternalInput")
    dstp = nc.dram_tensor("dstp", [128, NPAIR * 16], i32, kind="ExternalInput")
    odstp = nc.dram_tensor("odstp", [128, NPAIR_OR * 16], i32, kind="ExternalInput")
    wenc = nc.dram_tensor("wenc", [128, 128], f32, kind="ExternalInput")
    benc = nc.dram_tensor("benc", [128, 1], f32, kind="ExternalInput")
    wnei = nc.dram_tensor("wnei", [L * 128, 128], f32, kind="ExternalInput")
    wroot = nc.dram_tensor("wroot", [L * 128, 128], f32, kind="ExternalInput")
    wsnei = nc.dram_tensor("wsnei", [L * 128, 128], f32, kind="ExternalInput")
    wsroot = nc.dram_tensor("wsroot", [L * 128, 128], f32, kind="ExternalInput")
    gb = nc.dram_tensor("gb", [128, 4 * L], f32, kind="ExternalInput")
    w1 = nc.dram_tensor("w1", [128, 256], f32, kind="ExternalInput")
    b1 = nc.dram_tensor("b1", [128, 2], f32, kind="ExternalInput")
    w2 = nc.dram_tensor("w2", [256, 10], f32, kind="ExternalInput")
    b2 = nc.dram_tensor("b2", [10, 1], f32, kind="ExternalInput")
    out = nc.dram_tensor("out", [G, T], f32, kind="ExternalOutput")

    def bc(tap, free_dims):
        """AP over tile `tap` with custom free dims (partition dim kept)."""
        return bass.AP(tensor=tap.tensor, offset=tap.offset,
                       ap=[list(tap.ap[0])] + [list(d) for d in free_dims])

    with tile.TileContext(nc) as tc, ExitStack() as ctx:
        pers = ctx.enter_context(tc.tile_pool(name="pers", bufs=1))
        share = ctx.enter_context(tc.tile_pool(name="share", bufs=1))
        hwp = ctx.enter_context(tc.tile_pool(name="hw", bufs=8))
        ohp = ctx.enter_context(tc.tile_pool(name="oh", bufs=6))
        scr = ctx.enter_context(tc.tile_pool(name="scr", bufs=2))
        stp = ctx.enter_context(tc.tile_pool(name="st", bufs=6))
        pwin = ctx.enter_context(tc.tile_pool(name="pwin", bufs=3, space="PSUM"))
        pmini = ctx.enter_context(tc.tile_pool(name="pmini", bufs=3, space="PSUM"))
        pa = ctx.enter_context(tc.tile_pool(name="pa", bufs=2, space="PSUM"))
        drp = ctx.enter_context(tc.tile_pool(name="dram", bufs=2, space="DRAM"))

        # ---- constants ----
        io32 = pers.tile([128, 128], i32, tag="io32")
        nc.gpsimd.iota(io32[:], pattern=[[1, 128]], base=0, channel_multiplier=0)
        iobf = pers.tile([128, 128], bf16, tag="iobf")
        nc.vector.tensor_copy(iobf[:], io32[:])
        ioc32 = pers.tile([128, 1], i32, tag="ioc32")
        nc.gpsimd.iota(ioc32[:], pattern=[[0, 1]], base=0, channel_multiplier=1)
        iocf = pers.tile([128, 1], f32, tag="iocf")
        nc.vector.tensor_copy(iocf[:], ioc32[:])
        idbf = pers.tile([128, 128], bf16, tag="idbf")
        nc.vector.tensor_tensor(out=idbf[:], in0=iobf[:],
                                in1=iocf[:, 0:1].to_broadcast([128, 128]),
                                op=Alu.is_equal)
        epst = pers.tile([128, 1], f32, tag="epst")
        nc.vector.memset(epst[:], EPS)

        benc_s = pers.tile([128, 1], f32, tag="benc")
        nc.sync.dma_start(out=benc_s[:], in_=benc[:])
        gb_s = pers.tile([128, 4 * L], f32, tag="gb")
        nc.sync.dma_start(out=gb_s[:], in_=gb[:])
        wenc_s = pers.tile([128, 128], f32, tag="wenc")
        nc.sync.dma_start(out=wenc_s[:], in_=wenc[:])
        wl = {}
        for nm, dram in (("wnei", wnei), ("wroot", wroot),
                         ("wsnei", wsnei), ("wsroot", wsroot)):
            for i in range(L):
                t = pers.tile([128, 128], f32, tag=f"{nm}{i}")
                nc.sync.dma_start(out=t[:], in_=dram[i * 128:(i + 1) * 128, :])
                wl[(nm, i)] = t
        w1_s = pers.tile([128, 256], f32, tag="w1")
        nc.sync.dma_start(out=w1_s[:], in_=w1[:])
        b1_s = pers.tile([128, 2], f32, tag="b1")
        nc.sync.dma_start(out=b1_s[:], in_=b1[:])
        w2a_s = pers.tile([128, 10], f32, tag="w2a")
        nc.sync.dma_start(out=w2a_s[:], in_=w2[0:128, :])
        w2b_s = pers.tile([128, 10], f32, tag="w2b")
        nc.sync.dma_start(out=w2b_s[:], in_=w2[128:256, :])
        b2_s = pers.tile([10, 1], f32, tag="b2")
        nc.sync.dma_start(out=b2_s[:], in_=b2[:])

        # ---- adjacency build:  a2t[pair][src, dst] = #edges(src -> dst) ----
        dsti = pers.tile([128, NPAIR * 16], i32, tag="dsti")
        nc.sync.dma_start(out=dsti[:], in_=dstp[:])
        dstbf = pers.tile([128, NPAIR * 16], f32, tag="dstbf")
        nc.vector.tensor_copy(dstbf[:], dsti[:])
        odsti = pers.tile([128, NPAIR_OR * 16], i32, tag="odsti")
        nc.sync.dma_start(out=odsti[:], in_=odstp[:])
        odstbf = pers.tile([128, NPAIR_OR * 16], f32, tag="odstbf")
        nc.vector.tensor_copy(odstbf[:], odsti[:])

        a2t = pers.tile([128, NPAIR * 128], f32, tag="a2t")
        a2tor = pers.tile([128, NPAIR_OR * 128], f32, tag="a2tor")

        def build_adj(dest_tile, src_bf, npair):
            for p in range(npair):
                pacc = pa.tile([128, 128], f32, tag="pa")
                for k in range(16):
                    oh = ohp.tile([128, 128], bf16, tag="oh")
                    # tensor_tensor (not tensor_scalar): TSPtr can't carry the
                    # two sync waits walrus needs to encode here
                    nc.vector.tensor_tensor(
                        out=oh[:], in0=iobf[:],
                        in1=src_bf[:, p * 16 + k:p * 16 + k + 1].to_broadcast([128, 128]),
                        op=Alu.is_equal)
                    nc.tensor.matmul(out=pacc[:], lhsT=idbf[:], rhs=oh[:],
                                     start=(k == 0), stop=(k == 15))
                nc.scalar.activation(out=dest_tile[:, p * 128:(p + 1) * 128],
                                     in_=pacc[:], func=Act.Copy)

        build_adj(a2t, dstbf, NPAIR)
        build_adj(a2tor, odstbf, NPAIR_OR)

        # ---- encoder: h = W_enc^T @ x^T + b_enc ----
        x_s = share.tile([128, NN], f32, tag="big")
        for (o, w) in WINS:
            nc.sync.dma_start(out=x_s[:, o:o + w], in_=xT[:, o:o + w])
        h_a = pers.tile([128, NN], f32, tag="ha")
        h_b = pers.tile([128, NN], f32, tag="hb")
        for (o, w) in WINS:
            cw = pwin.tile([128, 512], f32, tag="pw")
            nc.tensor.matmul(out=cw[:, 0:w], lhsT=wenc_s[:], rhs=x_s[:, o:o + w],
                             start=True, stop=True)
            nc.scalar.activation(out=h_a[:, o:o + w], in_=cw[:, 0:w],
                                 func=Act.Identity, bias=benc_s[:, 0:1])

        c2T = pers.tile([128, NOR], f32, tag="c2")
        xsum = pers.tile([128, NOR], f32, tag="xsum")
        b2t = pers.tile([128, NOR], f32, tag="b2t")

        hbufs = [h_a, h_b]
        for li in range(L):
            h_cur = hbufs[li % 2]
            h_new = hbufs[(li + 1) % 2]
            c1T = share.tile([128, NN], f32, tag="big")

            # x_sum^T = sum over the S subgraph copies (1/S folded into weights)
            nc.vector.tensor_reduce(
                out=xsum[:],
                in_=bc(h_cur[:], [[S * N0, G], [1, N0], [N0, S]]),
                axis=mybir.AxisListType.X, op=Alu.add)

            ns1 = stp.tile([128, len(WINS)], f32, tag="s1c")
            nq1 = stp.tile([128, len(WINS)], f32, tag="q1c")
            ns2 = stp.tile([128, len(WINS_OR)], f32, tag="s2c")
            nq2 = stp.tile([128, len(WINS_OR)], f32, tag="q2c")

            # conv1 = h @ W_root + A @ (h @ W_nei)   (feature-partitioned out)
            for wi, (o, w) in enumerate(WINS):
                hw_t = []
                for q in range(w // 128):
                    pr = o + q * 128
                    pm = pmini.tile([128, 128], f32, tag="pm")
                    nc.tensor.matmul(out=pm[:], lhsT=h_cur[:, pr:pr + 128],
                                     rhs=wl[("wnei", li)][:], start=True, stop=True)
                    hwt = hwp.tile([128, 128], f32, tag="hw")
                    nc.scalar.activation(out=hwt[:], in_=pm[:], func=Act.Copy)
                    hw_t.append(hwt)
                cw = pwin.tile([128, 512], f32, tag="pw")
                nc.tensor.matmul(out=cw[:, 0:w], lhsT=wl[("wroot", li)][:],
                                 rhs=h_cur[:, o:o + w], start=True, stop=True)
                for q in range(w // 128):
                    gp = (o + q * 128) // 128
                    nc.tensor.matmul(out=cw[:, q * 128:(q + 1) * 128],
                                     lhsT=hw_t[q][:],
                                     rhs=a2t[:, gp * 128:(gp + 1) * 128],
                                     start=False, stop=True,
                                     skip_group_check=True)
                nc.scalar.activation(out=c1T[:, o:o + w], in_=cw[:, 0:w],
                                     func=Act.Copy, accum_out=ns1[:, wi:wi + 1])
                sq = scr.tile([128, 512], f32, tag="sq")
                nc.scalar.activation(out=sq[:, 0:w], in_=c1T[:, o:o + w],
                                     func=Act.Square, accum_out=nq1[:, wi:wi + 1])

            # conv2 on original graph (input x_sum, pre-scaled weights)
            for wi, (o, w) in enumerate(WINS_OR):
                hw_t = []
                for q in range(w // 128):
                    pr = o + q * 128
                    pm = pmini.tile([128, 128], f32, tag="pm")
                    nc.tensor.matmul(out=pm[:], lhsT=xsum[:, pr:pr + 128],
                                     rhs=wl[("wsnei", li)][:], start=True, stop=True)
                    hwt = hwp.tile([128, 128], f32, tag="hw")
                    nc.scalar.activation(out=hwt[:], in_=pm[:], func=Act.Copy)
                    hw_t.append(hwt)
                cw = pwin.tile([128, 512], f32, tag="pw")
                nc.tensor.matmul(out=cw[:, 0:w], lhsT=wl[("wsroot", li)][:],
                                 rhs=xsum[:, o:o + w], start=True, stop=True)
                for q in range(w // 128):
                    gp = (o + q * 128) // 128
                    nc.tensor.matmul(out=cw[:, q * 128:(q + 1) * 128],
                                     lhsT=hw_t[q][:],
                                     rhs=a2tor[:, gp * 128:(gp + 1) * 128],
                                     start=False, stop=True,
                                     skip_group_check=True)
                nc.scalar.activation(out=c2T[:, o:o + w], in_=cw[:, 0:w],
                                     func=Act.Copy, accum_out=ns2[:, wi:wi + 1])
                sq = scr.tile([128, 512], f32, tag="sq")
                nc.scalar.activation(out=sq[:, 0:w], in_=c2T[:, o:o + w],
                                     func=Act.Square, accum_out=nq2[:, wi:wi + 1])

            # pack per-core partial sums -> AllReduce -> global BN stats
            pack = stp.tile([128, 4], f32, tag="pack")
            nc.vector.tensor_reduce(out=pack[:, 0:1], in_=ns1[:],
                                    axis=mybir.AxisListType.X, op=Alu.add)
            nc.vector.tensor_reduce(out=pack[:, 1:2], in_=nq1[:],
                                    axis=mybir.AxisListType.X, op=Alu.add)
            nc.vector.tensor_reduce(out=pack[:, 2:3], in_=ns2[:],
                                    axis=mybir.AxisListType.X, op=Alu.add)
            nc.vector.tensor_reduce(out=pack[:, 3:4], in_=nq2[:],
                                    axis=mybir.AxisListType.X, op=Alu.add)

            if n_cores > 1:
                cc_in = drp.tile([128, 4], f32, tag="ccin")
                cc_out = drp.tile([128, 4], f32, tag="ccout")
                nc.sync.dma_start(out=cc_in[:], in_=pack[:])
                nc.gpsimd.collective_compute(
                    "AllReduce", Alu.add,
                    replica_groups=[list(range(n_cores))],
                    ins=[cc_in[:].opt()], outs=[cc_out[:].opt()])
                gst = stp.tile([128, 4], f32, tag="gst")
                nc.sync.dma_start(out=gst[:], in_=cc_out[:])
            else:
                gst = pack

            # BN affine params: s = gamma*rstd, t = beta - mu*s
            sts = []
            for bi, (ccol, cnt) in enumerate(((0, NG1), (2, NG2))):
                mu = stp.tile([128, 1], f32, tag="mu")
                nc.vector.tensor_scalar(out=mu[:], in0=gst[:, ccol:ccol + 1],
                                        scalar1=1.0 / cnt, scalar2=None, op0=Alu.mult)
                ex2 = stp.tile([128, 1], f32, tag="ex2")
                nc.vector.tensor_scalar(out=ex2[:], in0=gst[:, ccol + 1:ccol + 2],
                                        scalar1=1.0 / cnt, scalar2=None, op0=Alu.mult)
                var = stp.tile([128, 1], f32, tag="var")
                nc.vector.tensor_tensor(out=var[:], in0=mu[:], in1=mu[:], op=Alu.mult)
                nc.vector.tensor_tensor(out=var[:], in0=ex2[:], in1=var[:],
                                        op=Alu.subtract)
                sd = stp.tile([128, 1], f32, tag="sd")
                nc.scalar.activation(out=sd[:], in_=var[:], func=Act.Sqrt,
                                     bias=epst[:, 0:1])
                rstd = stp.tile([128, 1], f32, tag="rstd")
                nc.vector.reciprocal(out=rstd[:], in_=sd[:])
                gcol = 4 * li + 2 * bi
                sv = stp.tile([128, 1], f32, tag="sv")
                nc.vector.tensor_tensor(out=sv[:], in0=gb_s[:, gcol:gcol + 1],
                                        in1=rstd[:], op=Alu.mult)
                tv = stp.tile([128, 1], f32, tag="tv")
                nc.vector.tensor_tensor(out=tv[:], in0=mu[:], in1=sv[:], op=Alu.mult)
                nc.vector.tensor_tensor(out=tv[:], in0=gb_s[:, gcol + 1:gcol + 2],
                                        in1=tv[:], op=Alu.subtract)
                sts.append((sv, tv))
            (s1v, t1v), (s2v, t2v) = sts

            # h_new = relu( (c1*s1 + t1) + (c2*s2 + t2)[node_idx broadcast] )
            nc.vector.tensor_scalar(out=b2t[:], in0=c2T[:], scalar1=s2v[:, 0:1],
                                    scalar2=t2v[:, 0:1], op0=Alu.mult, op1=Alu.add)
            nc.vector.tensor_scalar(out=h_new[:], in0=c1T[:], scalar1=s1v[:, 0:1],
                                    scalar2=t1v[:, 0:1], op0=Alu.mult, op1=Alu.add)
            nc.vector.tensor_tensor(
                out=h_new[:], in0=h_new[:],
                in1=bc(b2t[:], [[N0, G], [0, S], [1, N0]]), op=Alu.add)
            nc.scalar.activation(out=h_new[:], in_=h_new[:], func=Act.Relu)

        # ---- pooling + MLP head ----
        h_fin = hbufs[L % 2]
        hg = stp.tile([128, G], f32, tag="hg")
        nc.vector.tensor_reduce(out=hg[:], in_=bc(h_fin[:], [[S * N0, G], [1, S * N0]]),
                                axis=mybir.AxisListType.X, op=Alu.add)
        z1s = []
        for half in range(2):
            zp = pmini.tile([128, G], f32, tag="pm")
            nc.tensor.matmul(out=zp[:], lhsT=w1_s[:, half * 128:(half + 1) * 128],
                             rhs=hg[:], start=True, stop=True)
            zs = hwp.tile([128, G], f32, tag="hw")
            nc.scalar.activation(out=zs[:], in_=zp[:], func=Act.Relu,
                                 bias=b1_s[:, half:half + 1])
            z1s.append(zs)
        z2 = pmini.tile([10, G], f32, tag="pm")
        nc.tensor.matmul(out=z2[:], lhsT=w2a_s[:], rhs=z1s[0][:], start=True,
                         stop=True)
        nc.tensor.matmul(out=z2[:], lhsT=w2b_s[:], rhs=z1s[1][:], start=False,
                         stop=True, skip_group_check=True)
        outs = hwp.tile([10, G], f32, tag="outs")
        nc.vector.tensor_scalar(out=outs[:], in0=z2[:], scalar1=b2_s[:, 0:1],
                                scalar2=None, op0=Alu.add)
        nc.sync.dma_start(out=out[:].rearrange("g t -> t g"), in_=outs[:])

    nc.compile()
    return nc


def shard_inputs(inputs, G, n_cores):
    """Validate structure and produce per-core in_maps."""
    x = np.asarray(inputs["x"])
    ei = np.asarray(inputs["edge_index"])
    oei = np.asarray(inputs["orig_edge_index"])
    NB = n_cores * G
    N = NB * S * N0
    NORIG = NB * N0
    E = N * DEG
    EORIG = NORIG * DEG
    assert x.shape == (N, D) and ei.shape == (2, E) and oei.shape == (2, EORIG)

    nid = np.arange(N)
    assert np.array_equal(np.asarray(inputs["batch"]), nid // (S * N0))
    assert np.array_equal(np.asarray(inputs["subgraph_batch"]), (nid // N0) % S)
    assert np.array_equal(np.asarray(inputs["subgraph_n_id"]), nid % N0)
    assert np.all(np.asarray(inputs["num_subgraphs"]) == S)
    assert np.all(np.asarray(inputs["num_nodes_per_subgraph"]) == N0)
    assert np.array_equal(np.asarray(inputs["subgraph_id_batch"]),
                          np.arange(NB * S) // S)
    assert np.array_equal(ei[0], np.repeat(nid, DEG))
    assert np.all(ei[1] // N0 == ei[0] // N0)
    assert np.array_equal(oei[0], np.repeat(np.arange(NORIG), DEG))
    assert np.all(oei[1] // N0 == oei[0] // N0)

    f32 = np.float32
    wnei = np.asarray(inputs["gnn_nei"]).reshape(L * D, D).astype(f32)
    wroot = np.asarray(inputs["gnn_root"]).reshape(L * D, D).astype(f32)
    wsnei = (np.asarray(inputs["gnn_sum_nei"]) / S).reshape(L * D, D).astype(f32)
    wsroot = (np.asarray(inputs["gnn_sum_root"]) / S).reshape(L * D, D).astype(f32)
    # conv biases cancel inside BatchNorm; gamma/beta packed per layer
    gbcols = []
    for i in range(L):
        gbcols += [np.asarray(inputs["bn_gamma"])[i], np.asarray(inputs["bn_beta"])[i],
                   np.asarray(inputs["bn_sum_gamma"])[i],
                   np.asarray(inputs["bn_sum_beta"])[i]]
    gbp = np.stack(gbcols, axis=1).astype(f32)
    w1p = (np.asarray(inputs["W1"]) / (S * N0)).astype(f32)   # fold graph-mean
    b1p = np.asarray(inputs["b1"]).reshape(2, D).T.astype(f32).copy()
    w2p = np.asarray(inputs["W2"]).astype(f32)
    b2p = np.asarray(inputs["b2"]).reshape(T, 1).astype(f32)
    bencp = np.asarray(inputs["b_enc"]).reshape(D, 1).astype(f32)
    wencp = np.asarray(inputs["W_enc"]).astype(f32)

    NN = G * S * N0
    NPAIR = NN // 128
    NPAIR_OR = (G * N0) // 128
    EC = NN * DEG
    EOC = G * N0 * DEG
    dst_rel = (ei[1] % 128).astype(np.int32)
    odst_rel = (oei[1] % 128).astype(np.int32)

    in_maps = []
    for c in range(n_cores):
        xTc = np.ascontiguousarray(x[c * NN:(c + 1) * NN].T.astype(f32))
        dc = dst_rel[c * EC:(c + 1) * EC]
        dstp = np.ascontiguousarray(
            dc.reshape(NPAIR, 128, 16).transpose(1, 0, 2).reshape(128, NPAIR * 16))
        oc = odst_rel[c * EOC:(c + 1) * EOC]
        odstp = np.ascontiguousarray(
            oc.reshape(NPAIR_OR, 128, 16).transpose(1, 0, 2).reshape(128, NPAIR_OR * 16))
        in_maps.append(dict(
            xT=xTc, dstp=dstp, odstp=odstp, wenc=wencp, benc=bencp,
            wnei=wnei, wroot=wroot, wsnei=wsnei, wsroot=wsroot, gb=gbp,
            w1=w1p, b1=b1p, w2=w2p, b2=b2p))
    return in_maps


TRACE = False          # set True to capture an NTFF profile on the next call
LAST_RESULTS = None    # BassKernelResults of the most recent call


def kernel(**inputs):
    global LAST_RESULTS
    from concourse import bass_utils

    G = B // N_CORES
    key = (G, N_CORES)
    if key not in _CACHE:
        _CACHE[key] = build_nc(G, N_CORES)
    nc = _CACHE[key]
    in_maps = shard_inputs(inputs, G, N_CORES)
    res = bass_utils.run_bass_kernel_spmd(
        nc, in_maps, core_ids=list(range(N_CORES)), trace=TRACE)
    LAST_RESULTS = res
    out = np.concatenate([res.results[c]["out"] for c in range(N_CORES)], axis=0)
    return out.astype(np.float32)
